# revision 1
# baseline (speedup 1.0000x reference)
"""Contrastive-loss kernel for Trainium2 (8 NeuronCores, SPMD data-parallel).

Math (from the reference):
    diag_A_is = (A_is_t + A_is_t_14 + A_is_t_28)[i, i, :]        # [B, D]
    diag_A_em = (A_em_t + A_em_t_14 + A_em_t_28)[i, i, :]        # [B, D]
    loss = sum_b relu( sum_d (0.4*m + 0.6*tr_m) * (diag_A_is - diag_A_em) )

Only the diagonals A[i, i, :] of the six [B, B, D] tensors are touched
(1/256th of the data).  Sharding strategy: batch-dim data parallel across
the 8 cores — the host gathers the diagonal rows (pure data movement) and
ships each core its 32 rows of the eight [B, D] operands packed into one
1.06 MB fp32 buffer; all arithmetic runs on-device.  Per-core partial
losses are summed on the host (8 scalars).

Device-side layout per core (SBUF tile xt [128 partitions x 2080 f32]):
  each [32, 1024] operand block is flattened row-major to [128, 256]
  (partition p = 4*row + quarter, 256 contiguous d's per partition).
  cols:  m 0:256 | tr 256:512 | E 512:544 | is0 544:800 | em0 800:1056 |
         is1 1056:1312 | em1 1312:1568 | is2a 1568:1696 | em2a 1696:1824 |
         is2b 1824:1952 | em2b 1952:2080
  E[p, b] = 1.0 iff p // 4 == b — used as the matmul rhs to sum the four
  per-partition quarter-row dots of each batch row (partition reduction).

Factoring: 0.4*m + 0.6*tr_m = 0.4 * (m + 1.5*tr_m) and
relu(0.4 x) = 0.4 relu(x), so the 0.4 is applied host-side to the scalar.

The DRAM input "x" is chunk-major (five contiguous [128, cols] blocks) so
every DMA reads one sequential DRAM range.  5 load DMAs spread over both
HWDGE rings (sync + scalar); DVE work is pipelined per chunk; each chunk's
per-partition dot lands in rowq_parts and is folded over partitions by
PSUM-accumulating 1-column matmuls against E; a final fused
relu+accumulate produces the scalar.

Raw bass (no TileContext) on purpose: this walrus build enforces a tiny
per-instruction sync-wait limit (Tile's kernel-tail Drain needs one wait
per live semaphore and fails codegen at 4), and Tile's epilogue barrier
costs several microseconds.  With explicit blocks every wait is its own
instruction.  Custom-DVE ops (tensor_tensor_reduce etc.) are avoided —
they lower to InstISA, which this walrus rejects ("ISA wrong length").
"""

import numpy as np

import concourse.bass as bass
import concourse.mybir as mybir
from concourse.bass_utils import run_bass_kernel_spmd

B = 256
D = 1024
N_CORES = 8
ROWS_PER_CORE = B // N_CORES  # 32
BLK = 256  # free-dim width of one packed [32, 1024] operand block
E_COLS = ROWS_PER_CORE  # 32
FREE = 8 * BLK + E_COLS  # 2080
O = 2 * BLK + E_COLS  # 544 = cols in chunk 0 (m, tr, E)
# chunk-major DRAM layout: chunk i is a contiguous [128, CHUNK_COLS[i]]
# block, spread over both HWDGE rings (sync + scalar) so transfers
# overlap and the DVE pipeline starts as soon as each chunk lands.
CHUNK_COLS = [O, 2 * BLK, 2 * BLK, BLK, BLK]
CHUNK_OFF = [0]
for _c in CHUNK_COLS:
    CHUNK_OFF.append(CHUNK_OFF[-1] + 128 * _c)

_NC_CACHE = None


def build_nc() -> bass.Bass:
    f32 = mybir.dt.float32
    Alu = mybir.AluOpType

    nc = bass.Bass()
    x = nc.dram_tensor("x", [128 * FREE], f32, kind="ExternalInput")
    out_d = nc.dram_tensor("out", [1, 1], f32, kind="ExternalOutput")

    def x_chunk(i):
        return x[CHUNK_OFF[i] : CHUNK_OFF[i + 1]].rearrange(
            "(p f) -> p f", f=CHUNK_COLS[i]
        )

    with (
        nc.sbuf_tensor("xt", [128, FREE], f32) as xt,
        nc.sbuf_tensor("w", [128, BLK], f32) as w,
        nc.sbuf_tensor("diff", [128, 3 * BLK], f32) as diff,
        nc.sbuf_tensor("prod", [128, 3 * BLK], f32) as prod,
        nc.sbuf_tensor("rowq_parts", [128, 4], f32) as rowq_parts,
        nc.sbuf_tensor("srelu", [1, E_COLS], f32) as srelu,
        nc.sbuf_tensor("total", [1, 1], f32) as total,
        nc.psum_tensor("ps", [1, E_COLS], f32) as ps,
        nc.semaphore("s1") as s1,  # sync ring: chunk0 load (+out store)
        nc.semaphore("s2") as s2,  # sync ring: chunk2 (is1 em1)
        nc.semaphore("a1") as a1,  # scalar ring: chunk1 (is0 em0)
        nc.semaphore("a2") as a2,  # scalar ring: chunk3 (is2a em2a)
        nc.semaphore("a3") as a3,  # scalar ring: chunk4 (is2b em2b)
        nc.semaphore("v_sem") as v_sem,
        nc.semaphore("pe_sem") as pe_sem,
        nc.Block() as block,
    ):
        m_ap = xt[:, 0:BLK]
        tr_ap = xt[:, BLK : 2 * BLK]
        e_ap = xt[:, 2 * BLK : O]
        # (is, em, w-slice, diff/prod col, width) per compute step
        steps = [
            (xt[:, 544:800], xt[:, 800:1056], w[:, :], 0, BLK),
            (xt[:, 1056:1312], xt[:, 1312:1568], w[:, :], BLK, BLK),
            (xt[:, 1568:1696], xt[:, 1696:1824], w[:, 0:128], 2 * BLK, 128),
            (xt[:, 1824:1952], xt[:, 1952:2080], w[:, 128:256], 2 * BLK + 128, 128),
        ]
        dma_waits = [(a1, 16), (s2, 16), (a2, 16), (a3, 16)]

        @block.sync
        def _(sync):
            sync.dma_start(out=xt[:, 0:O], in_=x_chunk(0)).then_inc(s1, 16)
            sync.dma_start(out=xt[:, 1056:1568], in_=x_chunk(2)).then_inc(s2, 16)
            sync.wait_ge(v_sem, 10)
            sync.dma_start(out=out_d[:], in_=total[:]).then_inc(s1, 16)
            sync.wait_ge(s1, 32)

        @block.scalar
        def _(scalar):
            scalar.dma_start(out=xt[:, 544:1056], in_=x_chunk(1)).then_inc(a1, 16)
            scalar.dma_start(out=xt[:, 1568:1824], in_=x_chunk(3)).then_inc(a2, 16)
            scalar.dma_start(out=xt[:, 1824:2080], in_=x_chunk(4)).then_inc(a3, 16)

        @block.vector
        def _(vector):
            # w = m + 1.5 * tr_m
            vector.wait_ge(s1, 16)
            nc.vector.scalar_tensor_tensor(
                out=w[:], in0=tr_ap, scalar=1.5, in1=m_ap,
                op0=Alu.mult, op1=Alu.add,
            ).then_inc(v_sem, 1)
            # per chunk: diff = is - em, then fused prod = diff * w with
            # accum_out = per-partition sum -> rowq_parts[:, i]
            for i, ((is_i, em_i, w_i, col, wd), dw) in enumerate(
                zip(steps, dma_waits)
            ):
                vector.wait_ge(*dw)
                nc.vector.tensor_sub(
                    diff[:, col : col + wd], is_i, em_i
                ).then_inc(v_sem, 1)
                vector.wait_ge(v_sem, 2 * i + 2)
                nc.vector.scalar_tensor_tensor(
                    out=prod[:, col : col + wd], in0=diff[:, col : col + wd],
                    scalar=1.0, in1=w_i, op0=Alu.mult, op1=Alu.mult,
                    accum_out=rowq_parts[:, i : i + 1],
                ).then_inc(v_sem, 1)
            # relu the 32 per-row sums (in PSUM), accumulate to one scalar
            vector.wait_ge(pe_sem, 1)
            nc.vector.tensor_scalar(
                out=srelu[:], in0=ps[:], scalar1=0.0, scalar2=None,
                op0=Alu.max, op1=Alu.add, accum_out=total[:],
            ).then_inc(v_sem, 1)

        @block.tensor
        def _(tensor):
            tensor.wait_ge(s1, 16)
            # ps[1, 32] += rowq_parts[:, i]^T @ E — PSUM-accumulate the four
            # chunk dots while folding each row's 4 partition-quarters
            for i in range(4):
                tensor.wait_ge(v_sem, 2 * i + 3)
                mm = nc.tensor.matmul(
                    ps[:], rowq_parts[:, i : i + 1], e_ap,
                    start=(i == 0), stop=(i == 3),
                )
                if i == 3:
                    mm.then_inc(pe_sem, 1)

    return nc


def pack_inputs(A_is_t, A_is_t_14, A_is_t_28, A_em_t, A_em_t_14, A_em_t_28, m, tr_m):
    idx = np.arange(B)

    def diag(a):
        return np.asarray(a)[idx, idx]  # [B, D] gather of the used diagonal

    def blk(a):  # per-core [128, 256] flattening of a [B, D] operand
        return np.asarray(a, dtype=np.float32).reshape(N_CORES, 128, BLK)

    is2 = blk(diag(A_is_t_28))
    em2 = blk(diag(A_em_t_28))
    X = np.empty((N_CORES, 128, FREE), dtype=np.float32)
    X[:, :, 0:BLK] = blk(m)
    X[:, :, BLK : 2 * BLK] = blk(tr_m)
    X[:, :, 2 * BLK : O] = np.repeat(np.eye(E_COLS, dtype=np.float32), 4, axis=0)
    X[:, :, 544:800] = blk(diag(A_is_t))
    X[:, :, 800:1056] = blk(diag(A_em_t))
    X[:, :, 1056:1312] = blk(diag(A_is_t_14))
    X[:, :, 1312:1568] = blk(diag(A_em_t_14))
    X[:, :, 1568:1696] = is2[:, :, :128]
    X[:, :, 1696:1824] = em2[:, :, :128]
    X[:, :, 1824:1952] = is2[:, :, 128:]
    X[:, :, 1952:2080] = em2[:, :, 128:]
    # chunk-major flat layout: each DMA reads one contiguous DRAM range
    bounds = [0, O, 1056, 1568, 1824, FREE]
    return [
        {
            "x": np.concatenate(
                [X[c, :, bounds[i] : bounds[i + 1]].ravel() for i in range(5)]
            )
        }
        for c in range(N_CORES)
    ]


def run(in_maps, **kwargs):
    global _NC_CACHE
    if _NC_CACHE is None:
        _NC_CACHE = build_nc()
    return run_bass_kernel_spmd(
        _NC_CACHE, in_maps, core_ids=list(range(N_CORES)), **kwargs
    )


def kernel(**inputs) -> np.ndarray:
    res = run(pack_inputs(**inputs))
    total = 0.4 * sum(float(r["out"][0, 0]) for r in res.results)
    return np.array([total], dtype=np.float32)



# revision 5
# speedup vs baseline: 1.0197x; 1.0197x over previous
"""Contrastive-loss kernel for Trainium2 (8 NeuronCores, SPMD data-parallel).

Math (from the reference):
    diag_A_is = (A_is_t + A_is_t_14 + A_is_t_28)[i, i, :]        # [B, D]
    diag_A_em = (A_em_t + A_em_t_14 + A_em_t_28)[i, i, :]        # [B, D]
    loss = sum_b relu( sum_d (0.4*m + 0.6*tr_m) * (diag_A_is - diag_A_em) )

Only the diagonals A[i, i, :] of the six [B, B, D] tensors are touched
(1/256th of the data).  Sharding strategy: batch-dim data parallel across
the 8 cores — the host gathers the diagonal rows (pure data movement) and
ships each core its 32 rows of the eight [B, D] operands as bf16; all
arithmetic runs on-device.  Per-core partial losses are summed on the host
(8 scalars).

bf16 packing halves HBM traffic (tolerance is 2e-2; bf16 rounding error on
the dots is ~4e-3), doubles DVE element rate, and makes the PE fold a
single-pass matmul (fp32 PE needs LOW/HIGH double pumping).

Device-side layout per core, two SBUF tiles:
  wt [128, 512]  = m 0:256 | tr_m 256:512          (from DMA "xw", 1 KB rows)
  at [128, 1568] = is0|em0|is1|em1|is2|em2 (6x256) | E 1536:1568
                                                   (from DMA "xa", 3136 B rows)
  each [32, 1024] operand block is flattened row-major to [128, 256]
  (partition p = 4*row + quarter, 256 contiguous d's per partition).
  E[p, b] = 1.0 iff p // 4 == b — matmul rhs that folds the four
  per-partition quarter-row dots of each batch row (partition reduction).

Factoring: 0.4*m + 0.6*tr_m = 0.4 * (m + 1.5*tr_m) and
relu(0.4 x) = 0.4 relu(x), so the 0.4 is applied host-side to the scalar.
Linearity: sum_d w*(is-em) = sum_d [is0|em0|...] * [w|-w|w|-w|w|-w], so one
scalar_tensor_tensor over the whole A block with a stride-0 (broadcast)
[w|-w] operand computes all six dot contributions in a single accumulator
pass (rowq[p] = per-partition quarter dot).

Pipeline: two big contiguous DMAs (one per HWDGE ring: sync=wt,
scalar=at) — large per-partition descriptors put SDMA near line rate.
DVE computes w/-w under the A transfer, then the fused pass; PE folds
partitions; the Scalar engine does relu+sum (activation accum) and
issues the 4-byte result store on the same engine, in order.

Raw bass (no TileContext) on purpose: this walrus build enforces a tiny
per-instruction sync-wait limit (Tile's kernel-tail Drain needs one wait
per live semaphore and fails codegen at 4), and Tile's epilogue barrier
costs several microseconds.  With explicit blocks every wait is its own
instruction.
"""

import numpy as np
import ml_dtypes

import concourse.bass as bass
import concourse.mybir as mybir
from concourse.bass_utils import run_bass_kernel_spmd

B = 256
D = 1024
N_CORES = 8
ROWS_PER_CORE = B // N_CORES  # 32
BLK = 256  # free-dim width of one packed [32, 1024] operand block
E_COLS = ROWS_PER_CORE  # 32
FREE_W = 2 * BLK  # 512: m | tr
FREE_A = 6 * BLK + E_COLS  # 1568: 6 A blocks | E

_NC_CACHE = None


def build_nc() -> bass.Bass:
    f32 = mybir.dt.float32
    bf16 = mybir.dt.bfloat16
    Alu = mybir.AluOpType
    Act = mybir.ActivationFunctionType

    nc = bass.Bass()
    xw = nc.dram_tensor("xw", [128 * FREE_W], bf16, kind="ExternalInput")
    xa = nc.dram_tensor("xa", [128 * FREE_A], bf16, kind="ExternalInput")
    out_d = nc.dram_tensor("out", [1, 1], f32, kind="ExternalOutput")

    with (
        nc.sbuf_tensor("wt", [128, FREE_W], bf16) as wt,
        nc.sbuf_tensor("at", [128, FREE_A], bf16) as at,
        nc.sbuf_tensor("w2", [128, 2 * BLK], bf16) as w2,
        nc.sbuf_tensor("prod", [128, 6 * BLK], bf16) as prod,
        nc.sbuf_tensor("rowq", [128, 1], bf16) as rowq,
        nc.sbuf_tensor("srelu", [1, E_COLS], f32) as srelu,
        nc.sbuf_tensor("total", [1, 1], f32) as total,
        nc.psum_tensor("ps", [1, E_COLS], f32) as ps,
        nc.semaphore("sw") as sw,  # sync ring: wt load
        nc.semaphore("sa") as sa,  # scalar ring: at load (+16) and out store (+16)
        nc.semaphore("vs") as vs,  # vector: fused dot pass done
        nc.semaphore("pe") as pe,  # tensor: partition fold done
        nc.Block() as block,
    ):
        m_ap = wt[:, 0:BLK]
        tr_ap = wt[:, BLK : 2 * BLK]
        a3 = at[:, 0 : 6 * BLK].rearrange("p (c f) -> p c f", f=2 * BLK)
        e_ap = at[:, 6 * BLK : FREE_A]
        prod3 = prod[:, :].rearrange("p (c f) -> p c f", f=2 * BLK)
        # [w | -w] repeated 3x via stride-0 outer dim
        w_b = w2[:, :].unsqueeze(1).broadcast_to([128, 3, 2 * BLK])

        @block.sync
        def _(sync):
            sync.dma_start(
                out=wt[:, :], in_=xw[:].rearrange("(p f) -> p f", f=FREE_W)
            ).then_inc(sw, 16)

        @block.scalar
        def _(scalar):
            scalar.dma_start(
                out=at[:, :], in_=xa[:].rearrange("(p f) -> p f", f=FREE_A)
            ).then_inc(sa, 16)
            # relu the 32 per-row dots (in PSUM) and fold to one scalar;
            # the result store is issued from this same engine, in order.
            scalar.wait_ge(pe, 1)
            nc.scalar.activation(
                out=srelu[:], in_=ps[:], func=Act.Relu, accum_out=total[:]
            ).then_inc(vs, 1)
            scalar.wait_ge(vs, 4)  # total committed (engines pipeline)
            scalar.dma_start(out=out_d[:], in_=total[:]).then_inc(sa, 16)
            scalar.wait_ge(sa, 32)

        @block.vector
        def _(vector):
            # w = m + 1.5*tr_m  and  -w = (-1.5)*tr_m - m
            vector.wait_ge(sw, 16)
            nc.vector.scalar_tensor_tensor(
                out=w2[:, 0:BLK], in0=tr_ap, scalar=1.5, in1=m_ap,
                op0=Alu.mult, op1=Alu.add,
            ).then_inc(vs, 1)
            nc.vector.scalar_tensor_tensor(
                out=w2[:, BLK : 2 * BLK], in0=tr_ap, scalar=-1.5, in1=m_ap,
                op0=Alu.mult, op1=Alu.subtract,
            ).then_inc(vs, 1)
            # fused dot: prod = A * [w|-w|w|-w|w|-w]; rowq = per-partition sum
            vector.wait_ge(vs, 2)  # w2 committed (DVE pipelines)
            vector.wait_ge(sa, 16)
            nc.vector.scalar_tensor_tensor(
                out=prod3, in0=a3, scalar=1.0, in1=w_b,
                op0=Alu.mult, op1=Alu.mult, accum_out=rowq[:, 0:1],
            ).then_inc(vs, 1)

        @block.tensor
        def _(tensor):
            # ps[1, 32] = rowq^T @ E — fold each row's 4 partition-quarters
            tensor.wait_ge(vs, 3)
            nc.tensor.matmul(
                ps[:], rowq[:, 0:1], e_ap, start=True, stop=True
            ).then_inc(pe, 1)

    return nc


def pack_inputs(A_is_t, A_is_t_14, A_is_t_28, A_em_t, A_em_t_14, A_em_t_28, m, tr_m):
    idx = np.arange(B)
    bf = ml_dtypes.bfloat16

    def blk(a):  # per-core [128, 256] flattening of a [B, D] operand, in bf16
        return np.ascontiguousarray(a, dtype=np.float32).astype(bf).reshape(
            N_CORES, 128, BLK
        )

    def dblk(a):  # diagonal gather then per-core flatten
        return blk(np.asarray(a)[idx, idx])

    Xw = np.empty((N_CORES, 128, FREE_W), dtype=bf)
    Xw[:, :, 0:BLK] = blk(m)
    Xw[:, :, BLK : 2 * BLK] = blk(tr_m)

    Xa = np.empty((N_CORES, 128, FREE_A), dtype=bf)
    Xa[:, :, 0 * BLK : 1 * BLK] = dblk(A_is_t)
    Xa[:, :, 1 * BLK : 2 * BLK] = dblk(A_em_t)
    Xa[:, :, 2 * BLK : 3 * BLK] = dblk(A_is_t_14)
    Xa[:, :, 3 * BLK : 4 * BLK] = dblk(A_em_t_14)
    Xa[:, :, 4 * BLK : 5 * BLK] = dblk(A_is_t_28)
    Xa[:, :, 5 * BLK : 6 * BLK] = dblk(A_em_t_28)
    Xa[:, :, 6 * BLK : FREE_A] = np.repeat(
        np.eye(E_COLS, dtype=np.float32), 4, axis=0
    ).astype(bf)

    return [
        {"xw": Xw[c].ravel(), "xa": Xa[c].ravel()} for c in range(N_CORES)
    ]


def run(in_maps, **kwargs):
    global _NC_CACHE
    if _NC_CACHE is None:
        _NC_CACHE = build_nc()
    return run_bass_kernel_spmd(
        _NC_CACHE, in_maps, core_ids=list(range(N_CORES)), **kwargs
    )


def kernel(**inputs) -> np.ndarray:
    res = run(pack_inputs(**inputs))
    total = 0.4 * sum(float(r["out"][0, 0]) for r in res.results)
    return np.array([total], dtype=np.float32)


# revision 7
# speedup vs baseline: 1.1671x; 1.1445x over previous
"""Contrastive-loss kernel for Trainium2 (8 NeuronCores, SPMD data-parallel).

Math (from the reference):
    diag_A_is = (A_is_t + A_is_t_14 + A_is_t_28)[i, i, :]        # [B, D]
    diag_A_em = (A_em_t + A_em_t_14 + A_em_t_28)[i, i, :]        # [B, D]
    loss = sum_b relu( sum_d (0.4*m + 0.6*tr_m) * (diag_A_is - diag_A_em) )

Only the diagonals A[i, i, :] of the six [B, B, D] tensors are touched
(1/256th of the data).  Sharding strategy: batch-dim data parallel across
the 8 cores — the host gathers the diagonal rows (pure data movement) and
ships each core its 32 rows of the eight [B, D] operands as bf16; all
arithmetic runs on-device.  Per-core partial losses are summed on the host
(8 scalars).

bf16 packing halves HBM traffic (tolerance is 2e-2; bf16 rounding error on
the dots is ~3e-4) and makes the PE fold a single-pass matmul (fp32 PE
needs LOW/HIGH double pumping).

Device-side layout per core:
  wt [128, 544]  = m 0:256 | tr_m 256:512 | E 512:544   (DMA "xw", 1088 B rows)
  at [128, 1536] = is0|em0 | is1|em1 | is2|em2          (DMA "xa", 3 chunks,
                                                         1024 B rows each)
  each [32, 1024] operand block is flattened row-major to [128, 256]
  (partition p = 4*row + quarter, 256 contiguous d's per partition).
  E[p, b] = 1.0 iff p // 4 == b — matmul rhs that folds the four
  per-partition quarter-row dots of each batch row (partition reduction).

Factoring: 0.4*m + 0.6*tr_m = 0.4 * (m + 1.5*tr_m) and
relu(0.4 x) = 0.4 relu(x), so the 0.4 is applied host-side to the scalar.
Linearity: sum_d w*(is-em) = sum_d [is|em] * [w|-w], so one
scalar_tensor_tensor per [is_i|em_i] chunk computes that chunk's dot
contribution in a single accumulator pass (rowq[:, i] = per-partition
quarter dots); the three chunks PSUM-accumulate through the E matmul.

Pipeline: W first (sync ring) so the DVE w/-w prep runs under the A
transfers (A1+A3 on the scalar ring, A2 behind W on the sync ring); each
chunk's fused dot starts as soon as that chunk lands.  relu+sum runs on
DVE (the Scalar engine's ACT path lazily loads a 1.3 us function table on
first use — measured, avoid).  The 4-byte result store is issued from the
sync ring.

Raw bass (no TileContext) on purpose: this walrus build enforces a tiny
per-instruction sync-wait limit (Tile's kernel-tail Drain needs one wait
per live semaphore and fails codegen at 4), and Tile's epilogue barrier
costs several microseconds.  With explicit blocks every wait is its own
instruction.  Engines pipeline, so a same-engine consumer of an earlier
op's output still needs a semaphore edge (the race detector enforces it).
"""

import numpy as np
import ml_dtypes

import concourse.bass as bass
import concourse.mybir as mybir
from concourse.bass_utils import run_bass_kernel_spmd

B = 256
D = 1024
N_CORES = 8
ROWS_PER_CORE = B // N_CORES  # 32
BLK = 256  # free-dim width of one packed [32, 1024] operand block
E_COLS = ROWS_PER_CORE  # 32
FREE_W = 2 * BLK + E_COLS  # 544: m | tr | E
FREE_A = 6 * BLK  # 1536: three [is|em] chunks
N_CHUNK = 3

_NC_CACHE = None


def build_nc() -> bass.Bass:
    f32 = mybir.dt.float32
    bf16 = mybir.dt.bfloat16
    Alu = mybir.AluOpType

    nc = bass.Bass()
    xw = nc.dram_tensor("xw", [128 * FREE_W], bf16, kind="ExternalInput")
    xa = nc.dram_tensor("xa", [128 * FREE_A], bf16, kind="ExternalInput")
    out_d = nc.dram_tensor("out", [1, 1], f32, kind="ExternalOutput")

    def xa_chunk(i):  # chunk-major flat layout: one contiguous DRAM range
        return xa[i * 128 * 2 * BLK : (i + 1) * 128 * 2 * BLK].rearrange(
            "(p f) -> p f", f=2 * BLK
        )

    with (
        nc.sbuf_tensor("wt", [128, FREE_W], bf16) as wt,
        nc.sbuf_tensor("at", [128, FREE_A], bf16) as at,
        nc.sbuf_tensor("w2", [128, 2 * BLK], bf16) as w2,
        nc.sbuf_tensor("prod", [128, FREE_A], bf16) as prod,
        nc.sbuf_tensor("rowq", [128, N_CHUNK], bf16) as rowq,
        nc.sbuf_tensor("srelu", [1, E_COLS], f32) as srelu,
        nc.sbuf_tensor("total", [1, 1], f32) as total,
        nc.psum_tensor("ps", [1, E_COLS], f32) as ps,
        nc.semaphore("sw") as sw,  # sync ring: wt load (+16), out store (+16)
        nc.semaphore("s1") as s1,  # scalar ring: chunk 0
        nc.semaphore("s2") as s2,  # sync ring: chunk 1
        nc.semaphore("s3") as s3,  # scalar ring: chunk 2
        nc.semaphore("vs") as vs,  # vector progress
        nc.semaphore("pe") as pe,  # tensor: partition fold done
        nc.Block() as block,
    ):
        m_ap = wt[:, 0:BLK]
        tr_ap = wt[:, BLK : 2 * BLK]
        e_ap = wt[:, 2 * BLK : FREE_W]
        chunk_sems = [s1, s2, s3]

        @block.sync
        def _(sync):
            sync.dma_start(
                out=wt[:, :], in_=xw[:].rearrange("(p f) -> p f", f=FREE_W)
            ).then_inc(sw, 16)
            sync.dma_start(out=at[:, 2 * BLK : 4 * BLK], in_=xa_chunk(1)).then_inc(
                s2, 16
            )
            sync.wait_ge(vs, 6)
            sync.dma_start(out=out_d[:], in_=total[:]).then_inc(sw, 16)
            sync.wait_ge(sw, 32)

        @block.scalar
        def _(scalar):
            scalar.dma_start(out=at[:, 0 : 2 * BLK], in_=xa_chunk(0)).then_inc(s1, 16)
            scalar.dma_start(out=at[:, 4 * BLK : 6 * BLK], in_=xa_chunk(2)).then_inc(
                s3, 16
            )

        @block.vector
        def _(vector):
            # w = m + 1.5*tr_m  and  -w = (-1.5)*tr_m - m
            vector.wait_ge(sw, 16)
            nc.vector.scalar_tensor_tensor(
                out=w2[:, 0:BLK], in0=tr_ap, scalar=1.5, in1=m_ap,
                op0=Alu.mult, op1=Alu.add,
            ).then_inc(vs, 1)
            nc.vector.scalar_tensor_tensor(
                out=w2[:, BLK : 2 * BLK], in0=tr_ap, scalar=-1.5, in1=m_ap,
                op0=Alu.mult, op1=Alu.subtract,
            ).then_inc(vs, 1)
            vector.wait_ge(vs, 2)  # w2 committed (engines pipeline)
            # per chunk: fused dot prod = [is|em] * [w|-w];
            # rowq[:, i] = per-partition sum
            for i in range(N_CHUNK):
                vector.wait_ge(chunk_sems[i], 16)
                nc.vector.scalar_tensor_tensor(
                    out=prod[:, 2 * BLK * i : 2 * BLK * (i + 1)],
                    in0=at[:, 2 * BLK * i : 2 * BLK * (i + 1)],
                    scalar=1.0, in1=w2[:, :],
                    op0=Alu.mult, op1=Alu.mult,
                    accum_out=rowq[:, i : i + 1],
                ).then_inc(vs, 1)
            # relu the 32 per-row dots (in PSUM), accumulate to one scalar
            vector.wait_ge(pe, 1)
            nc.vector.tensor_scalar(
                out=srelu[:], in0=ps[:], scalar1=0.0, scalar2=None,
                op0=Alu.max, op1=Alu.add, accum_out=total[:],
            ).then_inc(vs, 1)

        @block.tensor
        def _(tensor):
            # ps[1, 32] += rowq[:, i]^T @ E — PSUM-accumulate the three
            # chunk dots while folding each row's 4 partition-quarters
            for i in range(N_CHUNK):
                tensor.wait_ge(vs, 3 + i)
                mm = nc.tensor.matmul(
                    ps[:], rowq[:, i : i + 1], e_ap,
                    start=(i == 0), stop=(i == N_CHUNK - 1),
                )
                if i == N_CHUNK - 1:
                    mm.then_inc(pe, 1)

    return nc


def pack_inputs(A_is_t, A_is_t_14, A_is_t_28, A_em_t, A_em_t_14, A_em_t_28, m, tr_m):
    idx = np.arange(B)
    bf = ml_dtypes.bfloat16

    def blk(a):  # per-core [128, 256] flattening of a [B, D] operand, in bf16
        return np.ascontiguousarray(a, dtype=np.float32).astype(bf).reshape(
            N_CORES, 128, BLK
        )

    def dblk(a):  # diagonal gather then per-core flatten
        return blk(np.asarray(a)[idx, idx])

    Xw = np.empty((N_CORES, 128, FREE_W), dtype=bf)
    Xw[:, :, 0:BLK] = blk(m)
    Xw[:, :, BLK : 2 * BLK] = blk(tr_m)
    Xw[:, :, 2 * BLK : FREE_W] = np.repeat(
        np.eye(E_COLS, dtype=np.float32), 4, axis=0
    ).astype(bf)

    Xa = np.empty((N_CORES, 128, FREE_A), dtype=bf)
    Xa[:, :, 0 * BLK : 1 * BLK] = dblk(A_is_t)
    Xa[:, :, 1 * BLK : 2 * BLK] = dblk(A_em_t)
    Xa[:, :, 2 * BLK : 3 * BLK] = dblk(A_is_t_14)
    Xa[:, :, 3 * BLK : 4 * BLK] = dblk(A_em_t_14)
    Xa[:, :, 4 * BLK : 5 * BLK] = dblk(A_is_t_28)
    Xa[:, :, 5 * BLK : 6 * BLK] = dblk(A_em_t_28)

    # xa is chunk-major: each 128x512 chunk is one contiguous DRAM range
    return [
        {
            "xw": Xw[c].ravel(),
            "xa": np.concatenate(
                [Xa[c, :, 2 * BLK * i : 2 * BLK * (i + 1)].ravel()
                 for i in range(N_CHUNK)]
            ),
        }
        for c in range(N_CORES)
    ]


def run(in_maps, **kwargs):
    global _NC_CACHE
    if _NC_CACHE is None:
        _NC_CACHE = build_nc()
    return run_bass_kernel_spmd(
        _NC_CACHE, in_maps, core_ids=list(range(N_CORES)), **kwargs
    )


def kernel(**inputs) -> np.ndarray:
    res = run(pack_inputs(**inputs))
    total = 0.4 * sum(float(r["out"][0, 0]) for r in res.results)
    return np.array([total], dtype=np.float32)
